# revision 6
# baseline (speedup 1.0000x reference)
"""Per-batch covariance on 8 Trainium2 NeuronCores.

Full input  : inputs [32, 8192, 128] f32
Full output : cov    [32, 128, 128] f32   (divide-by-N covariance)

Sharding: pure data parallel — batch dim split 4 per core, no collectives.

Per-core math for each batch item X [N=8192, D=128]:
    cov = (X^T X - colsum colsum^T / N) / N

Design (v18; best ~59.5us, typical ~60-62us vs 88.8us baseline; the
input stream runs at the ~360 GB/s per-core HBM roofline, so the
kernel sits at the memory wall):
- DMA: each partition carries R=32 consecutive DRAM rows
  (n = 4096 s + 32 p + j). Each supergroup tile (2 MB) loads via two
  dma_starts of 16 j each -> fully contiguous 8 KiB descriptors on
  both sides and half-tile semaphore granularity so the PE tracks the
  stream closely. All input DMAs ride the SP HWDGE queue (one queue =
  sequential HBM addresses; a second queue thrashes row locality),
  outputs ride the ACT queue. The final supergroup splits into 4
  sub-DMAs: 4 KiB descriptors are the smallest that still saturate
  the bus (packet cost ~50ns + 32ns/KiB -> 16 engines x 22.9 GB/s >
  360 GB/s), while 8-chunk semaphore granularity keeps the end-of-
  stream PE drain ~1us. Smaller splits (2/1 KiB) measurably slow the
  last 2 MB; coarser ones stretch the drain.
- Early-start: after compile, _hoist_early_dmas moves the first four
  (wait-free) input DMAs into the entry block between SP\'s barrier-
  arrival signal and its release-wait, so the stream begins the moment
  SP\'s NEFF wrapper ends (~0.9us earlier) without delaying the
  barrier for the other engines.
- PE in bf16 via a zero-cost stride-2 view: bf16 is the high half of
  f32, so bitcast + stride-2 APs read the f32 tile as truncated bf16
  directly (no conversion pass). bf16 matmuls stream 1 row/cycle at
  ANY output width (f32r needs >= 256 wide), so each 128-row chunk
  costs one 128-wide matmul instead of the baseline\'s 384-wide f32r
  pair scheme (3x less PE streaming).
- colsum: per chunk a width-1 matmul (rhs = ones[128,1] bf16) into a
  separate PSUM tile. The lowering splits every matmul into
  InstLdweights + InstMatmult(ldweights=False); _dedup_ldweights drops
  the colsum matmul\'s duplicate weight load (identical weights AP as
  the preceding S-matmul), keeping the PE weight port at 128 rows per
  128-row chunk.
- Truncation debias: bf16-truncating both factors biases each product
  by ~2*2^-9*E[1/mantissa] ~ -0.56%; the final 1/N scale carries the
  first-order compensation (rel err 2.7e-4 vs 2e-2 gate).
- Mean correction: colsum column -> row via a bf16 identity matmul,
  then a K=1 rank-1 matmul accumulates -colsum colsum^T / N into the
  S PSUM; ACT does the scaled PSUM->SBUF copy and issues the output
  DMA (engine-local, no extra sem hop). For the LAST batch the
  column->row transpose would sit on the critical tail, so the column
  colsum stops 4 chunks early and the last 4 chunks OPEN the row-PSUM
  accumulation group directly via ones-stationary row matmuls
  (PSUM accumulation is order-agnostic); the bulk-colsum transpose
  CLOSES the group afterwards, so the ACT copy feeding it overlaps
  the tail chunks instead of stalling the in-order PE mid-drain. The
  end chain is just PE -> DVE copy/scale -> PE rank-1 -> ACT copy+DMA.
"""

import numpy as np

B, N, D = 32, 8192, 128
N_CORES = 8
B_PER = B // N_CORES   # 4 batch items per core

R = 32                 # consecutive DRAM rows per partition
SG = N // (128 * R)    # supergroups per batch
LAST_SPLIT = 4         # sub-DMAs for the final supergroup of the final batch

# bf16 truncation loses mantissa mass: E[x_trunc] ~ (1 - d) x with
# d ~ 2^-8 * E[1/m] ~ 0.0028 per factor; compensate both factors.
DEBIAS = 1.0 + 2 * 0.00282
SCALE = DEBIAS / N

_CACHE = {}


def _dedup_ldweights(nc):
    """Remove back-to-back duplicate InstLdweights (identical weights AP).

    The lowering splits every InstMatmult into InstLdweights +
    InstMatmult(ldweights=False). The width-1 colsum matmul reuses the
    exact weights the preceding S-matmul loaded, so its reload is pure
    weight-port waste. Safe to drop when the duplicate has no sync and
    no other PE instruction (self-loading matmul / ldweights) ran in
    between.
    """
    import concourse.mybir as mybir

    removed = 0
    for f in nc.m.functions:
        for blk in f.blocks:
            last_key = None
            keep = []
            for inst in blk.instructions:
                if isinstance(inst, mybir.InstLdweights):
                    key = str(inst.ins[0])
                    si = inst.sync_info
                    clean = si is None or (not si.on_wait and not si.on_update)
                    if key == last_key and clean and not inst.nosync_dependency_names():
                        removed += 1
                        continue
                    last_key = key
                elif isinstance(inst, mybir.InstMatmult):
                    if inst.ldweights is not False:
                        last_key = None  # self-loading matmul clobbers weights
                elif isinstance(inst, mybir.InstMatmultMx):
                    last_key = None
                keep.append(inst)
            blk.instructions = keep
    return removed


def _hoist_early_dmas(nc, k=4):
    """Move the first k wait-free SP input DMAs into the entry block,
    between SP's barrier-arrival signal and its release-wait. Their
    buffers are free and the source DRAM is populated before NEFF
    start, so the stream begins the moment SP's wrapper ends instead
    of after the barrier round-trip — without delaying the arrival
    signal the other engines block on."""
    import concourse.mybir as mybir

    f = nc.m.functions[0]
    entry, body = f.blocks[0], f.blocks[1]
    moved = []
    keep = []
    for inst in body.instructions:
        if (
            len(moved) < k
            and isinstance(inst, mybir.InstDMACopy)
            and inst.engine == mybir.EngineType.SP
            and (inst.sync_info is None or not inst.sync_info.on_wait)
        ):
            moved.append(inst)
            continue
        keep.append(inst)
    body.instructions = keep
    el = entry.instructions
    sp_evt = next(
        i
        for i, inst in enumerate(el)
        if inst.engine == mybir.EngineType.SP
        and isinstance(inst, mybir.InstEventSemaphore)
    )
    entry.instructions = el[:sp_evt] + moved + el[sp_evt:]
    return len(moved)


def _build_program():
    import concourse.bacc as bacc
    import concourse.mybir as mybir
    import concourse.tile as tile

    fp32 = mybir.dt.float32
    bf16 = mybir.dt.bfloat16
    nc = bacc.Bacc(None)

    x = nc.declare_dram_parameter("inputs", [B_PER, N, D], fp32, isOutput=False)
    out = nc.declare_dram_parameter("cov", [B_PER, D, D], fp32, isOutput=True)

    with tile.TileContext(nc) as tc:
        with (
            tc.tile_pool(name="xin", bufs=4) as xin,
            tc.tile_pool(name="acc", bufs=2, space="PSUM") as acc_pool,
            tc.tile_pool(name="cs", bufs=2, space="PSUM") as cs_pool,
            tc.tile_pool(name="rowp", bufs=2, space="PSUM") as rowp_pool,
            tc.tile_pool(name="small", bufs=8) as small,
            tc.tile_pool(name="const", bufs=1) as const,
            tc.tile_pool(name="outp", bufs=2) as outp,
        ):
            ident = const.tile([128, 128], bf16)
            nc.gpsimd.memset(ident[:], 1.0)
            nc.gpsimd.affine_select(
                ident[:],
                ident[:],
                pattern=[[-1, 128]],
                compare_op=mybir.AluOpType.is_equal,
                fill=0.0,
                base=0,
                channel_multiplier=1,
            )
            onesb = const.tile([128, 1], bf16)
            nc.gpsimd.memset(onesb[:], 1.0)
            # Warmup matmul reading only `ident`: absorbs the Pool-sem wait
            # so later PE instructions don't need it.
            warm = rowp_pool.tile([1, D], fp32, tag="rowp")
            nc.tensor.matmul(warm[:], ident[:, 0:1], ident[:])



            TAIL = 4  # trailing chunks whose colsum accumulates in row form

            for b in range(B_PER):
                last_b = b == B_PER - 1
                acc = acc_pool.tile([128, D], fp32, tag="acc")
                cs = cs_pool.tile([128, 1], fp32, tag="cs")
                rp_a = None
                for s in range(SG):
                    xt = xin.tile([128, R, D], fp32, tag="xin")
                    src = x[b, s * 128 * R : (s + 1) * 128 * R, :].rearrange(
                        "(p j) d -> p j d", p=128, j=R
                    )
                    last_tile = last_b and s == SG - 1
                    nsub = LAST_SPLIT if last_tile else 2
                    jstep = R // nsub
                    for sub in range(nsub):
                        js = slice(sub * jstep, (sub + 1) * jstep)
                        nc.sync.dma_start(xt[:, js, :], src[:, js, :])
                    xb = xt[:].bitcast(bf16).rearrange(
                        "p j (d two) -> p j d two", two=2
                    )
                    for j in range(R):
                        w = xb[:, j, :, 1]  # [128, 128] stride-2 bf16 view
                        first = s == 0 and j == 0
                        last = s == SG - 1 and j == R - 1
                        nc.tensor.matmul(acc[:], w, w, start=first, stop=last)
                        if not last_tile or j < R - TAIL:
                            # Column-form colsum for the bulk of the batch.
                            nc.tensor.matmul(
                                cs[:],
                                w,
                                onesb[:],
                                start=first,
                                stop=last_tile and j == R - TAIL - 1,
                            )
                        else:
                            # Tail chunks: accumulate colsum directly in ROW
                            # form (lhsT = ones loads once; dedup strips the
                            # repeats).
                            if rp_a is None:
                                rp_a = rowp_pool.tile([1, D], fp32, tag="rowp")
                            nc.tensor.matmul(
                                rp_a[:],
                                onesb[:],
                                w,
                                start=j == R - TAIL,
                                stop=False,
                                skip_group_check=True,
                            )

                # Mean correction + output for batch b.
                if last_b:
                    c_col = small.tile([128, 1], bf16)
                    nc.scalar.copy(c_col[:], cs[:])
                    nc.tensor.matmul(
                        rp_a[:],
                        c_col[:],
                        ident[:],
                        start=False,
                        stop=True,
                        skip_group_check=True,
                    )
                    c_row = small.tile([1, D], bf16)
                    nc.vector.tensor_copy(c_row[:], rp_a[:])
                    c_row_n = small.tile([1, D], bf16)
                    nc.vector.tensor_scalar_mul(c_row_n[:], rp_a[:], -1.0 / N)
                else:
                    c_col = small.tile([128, 1], bf16)
                    nc.scalar.copy(c_col[:], cs[:])
                    rp = rowp_pool.tile([1, D], fp32, tag="rowp")
                    nc.tensor.matmul(rp[:], c_col[:], ident[:])
                    c_row = small.tile([1, D], bf16)
                    nc.scalar.copy(c_row[:], rp[:])
                    c_row_n = small.tile([1, D], bf16)
                    nc.vector.tensor_scalar_mul(c_row_n[:], rp[:], -1.0 / N)
                nc.tensor.matmul(
                    acc[:],
                    c_row[:],
                    c_row_n[:],
                    start=False,
                    stop=True,
                    skip_group_check=True,
                )
                ot = outp.tile([128, D], fp32)
                nc.scalar.mul(ot[:], acc[:], SCALE)
                nc.scalar.dma_start(out[b], ot[:])

    ndup = _dedup_ldweights(nc)
    assert ndup >= 200, f"dedup removed only {ndup}"
    nc.compile()
    _hoist_early_dmas(nc)
    return nc


def _get_program():
    if "nc" not in _CACHE:
        _CACHE["nc"] = _build_program()
    return _CACHE["nc"]


def kernel(**inputs) -> np.ndarray:
    from concourse.bass_utils import run_bass_kernel_spmd

    x = np.asarray(inputs["inputs"], dtype=np.float32)
    assert x.shape == (B, N, D), x.shape

    nc = _get_program()
    in_maps = [
        {"inputs": np.ascontiguousarray(x[c * B_PER : (c + 1) * B_PER])}
        for c in range(N_CORES)
    ]
    res = run_bass_kernel_spmd(nc, in_maps, list(range(N_CORES)))
    return np.concatenate([res.results[c]["cov"] for c in range(N_CORES)], axis=0)
